# revision 25
# baseline (speedup 1.0000x reference)
"""Trainium2 Bass kernel for CrossModalAttention2d.

Reference computation (per batch element b):
    q = Wq @ face[b] + bq          # [64, 1024]   (face as [C=512, N=1024])
    k = Wk @ audio[b] + bk         # [64, 1024]
    v = Wv @ audio[b] + bv         # [512, 1024]
    attn = softmax(q^T k / 8, axis=-1)          # [1024, 1024]
    out = gamma * (v @ attn^T) + face[b]        # [512, 1024]

Distribution: data-parallel over batch B=32 across 8 NeuronCores
(4 batch elements per core); every core holds the full (small) weights.

Device-side design notes (v2 — software-pipelined):
- All heavy matmuls run in fp8 DoubleRow on TensorE; energy in bf16
  (K=64 row-packed pairs run concurrently in disjoint PE row halves).
- Energy is computed directly in TRANSPOSED layout ET[nk, nq] = k^T q,
  so the attention matrix is produced with nk on partitions — exactly
  the layout the PV matmul needs as its moving operand.
- softmax normalization: max-subtraction + clip(+-50) are numerical
  no-ops for this operator (energies are O(1)); exp(e/8) directly on
  ScalarE, normalize by column sums (ones-matmul + fast reciprocal).
- gamma is folded into Wv and bv ON HOST: Wv_scaled = gamma*Wv, and the
  residual input is face + gamma*bv in bf16 (v-bias passes through
  softmax exactly since attn rows sum to 1). The Vt PSUM->SBUF cast is
  a pure copy, split across ScalarE/VectorE.
- IO is slimmed: residual face in bf16 (not fp32), output in bf16
  (host upcasts) — halves the dominant DMA traffic.
- exp runs as FD=1024 activations over 2-bank PSUM tiles (halves the
  per-instruction overhead on ScalarE, the co-bottleneck engine).
- Residual adds run on the otherwise-idle GpSimd engine (except the
  last batch, where VectorE is used to minimize the serial tail).
- Software pipelining: batch b's energy matmuls are interleaved with
  batch b-1's PV matmuls in the emission (= priority) order, so the
  PE never waits on ScalarE's exp chain and the HAM clock stays warm.
"""

from contextlib import ExitStack

import ml_dtypes
import numpy as np

import concourse.bass as bass
import concourse.mybir as mybir
import concourse.tile as tile
from concourse import bacc
from concourse.bass import ds
from concourse.bass_utils import run_bass_kernel_spmd

N_CORES = 8
B = 32
C = 512
CQK = 64
N = 1024          # Nq = Nk = 32*32
H = W = 32
BPC = B // N_CORES  # batches per core
CC = C // 128       # 4 c-chunks
NT = N // 128       # 8 nk-tiles
NJ = N // 512       # 2 nq halves (PSUM bank = 512 fp32)

BF16 = mybir.dt.bfloat16
FP8 = mybir.dt.float8e4
F32 = mybir.dt.float32
DR = mybir.MatmulPerfMode.DoubleRow
EXP = mybir.ActivationFunctionType.Exp

_PROGRAM = None


class _BatchState:
    """SBUF tiles of one in-flight batch."""
    __slots__ = ("b", "face", "audio", "facer", "q", "k", "vt", "pt",
                 "recip", "sp")


def _emit_dma_in(nc, inpool, io, b):
    """Issue face/audio input DMAs for batch b (fp8 projection inputs).
    The bf16 residual input is DMAed separately (see _emit_dma_facer) so
    its slot-reuse wait can never sit ahead of the out-DMAs that free it
    in the in-order sync queue."""
    face8, audio8 = io["face8"], io["audio8"]
    st = _BatchState()
    st.b = b
    st.face = inpool.tile([128, CC, N], FP8, tag="face", name=f"face{b}")
    st.audio = inpool.tile([128, CC, N], FP8, tag="audio", name=f"audio{b}")
    # full-width rows (contiguous 1 KiB lines -> full HBM efficiency);
    # ScalarE carries no DMA descriptors at all: it is the exp engine and
    # must never be the resource the PE waits on
    for kk in range(CC):
        nc.sync.dma_start(st.face[:, kk, :], face8[b, kk])
        nc.sync.dma_start(st.audio[:, kk, :], audio8[b, kk])
    return st


def _emit_dma_facer(nc, inpool, io, st):
    # gpsimd queue: keeps the bf16 residual stream off the sync queue
    st.facer = inpool.tile([128, CC, N], BF16, tag="facer", name=f"facer{st.b}")
    for kk in range(CC):
        nc.gpsimd.dma_start(st.facer[:, kk, :], io["faceres"][st.b, kk])


def _emit(nc, tc, ctx, io):
    wpool = ctx.enter_context(tc.tile_pool(name="weights", bufs=1))
    inpool = ctx.enter_context(tc.tile_pool(name="inputs", bufs=2))
    qkpool = ctx.enter_context(tc.tile_pool(name="qk", bufs=2))
    vtpool = ctx.enter_context(tc.tile_pool(name="vt", bufs=2))
    ptpool = ctx.enter_context(tc.tile_pool(name="pt", bufs=2))
    misc = ctx.enter_context(tc.tile_pool(name="misc", bufs=2))
    tmppool = ctx.enter_context(tc.tile_pool(name="tmp", bufs=4))
    gps = ctx.enter_context(tc.tile_pool(name="gps", bufs=4, space="PSUM"))
    eps = ctx.enter_context(tc.tile_pool(name="eps", bufs=2, space="PSUM"))

    # --- persistent weights/constants ---
    # memsets first: the PE warm-up matmuls depend only on these
    ones_mat = wpool.tile([128, 2, 128], FP8)
    nc.vector.memset(ones_mat[:], 1.0)
    warm_rhs = wpool.tile([128, 2, 512], FP8)
    nc.vector.memset(warm_rhs[:], 1.0)
    warm_sb = wpool.tile([128, 1], F32)
    nc.vector.memset(warm_sb[:], 0.0)

    # ~3.5us of dummy matmuls on memset data: keeps the PE busy through
    # the HAM activity window during the initial input-DMA wait, so the
    # first real matmuls run at 2.4 GHz instead of the cold 1.2 GHz
    warm_mm = gps.tile([128, 512], F32, tag="g")
    for _ in range(8):
        nc.tensor.matmul(warm_mm[:], ones_mat[:], warm_rhs[:],
                         start=True, stop=True, perf_mode=DR)

    # warm the ScalarE exp table off the critical path
    warm_ps = gps.tile([128, 1], F32, tag="g")
    nc.scalar.activation(warm_ps[:], warm_sb[:], EXP)

    # weights on the gpsimd HW queue (parallel with face/audio streams);
    # wqk (needed by the first matmuls) is a separate small DMA so the
    # pipeline can start before wv lands
    wqk_sb = wpool.tile([128, CC, 256], FP8)
    nc.gpsimd.dma_start(wqk_sb[:], io["wqk"][:])
    wv_sb = wpool.tile([128, CC, C], FP8)  # pre-scaled by gamma on host
    nc.gpsimd.dma_start(wv_sb[:], io["wv"][:])
    WQ_OFF, WK_OFF = 0, 128
    bq_sb = wpool.tile([128, 1], F32)
    nc.gpsimd.dma_start(bq_sb[:], io["bq"][:])
    bk_sb = wpool.tile([128, 1], F32)
    nc.gpsimd.dma_start(bk_sb[:], io["bk"][:])

    out = io["out"]

    def emit_qk_proj(st):
        """q/k projections: [128, 1024] (dup halves) = [W|W] @ x."""
        b = st.b
        st.q = qkpool.tile([128, N], BF16, tag="q", name=f"q{b}")
        st.k = qkpool.tile([128, N], BF16, tag="k", name=f"k{b}")
        for (w_off, x, dst, bias) in ((WQ_OFF, st.face, st.q, bq_sb),
                                      (WK_OFF, st.audio, st.k, bk_sb)):
            for j in range(NJ):
                p = gps.tile([128, 512], F32, tag="g", name=f"qkp{b}_{j}")
                for kk in range(0, CC, 2):
                    nc.tensor.matmul(p[:], wqk_sb[:, kk:kk + 2, ds(w_off, 128)],
                                     x[:, kk:kk + 2, ds(j * 512, 512)],
                                     start=(kk == 0), stop=(kk == CC - 2),
                                     perf_mode=DR)
                nc.scalar.add(dst[:, ds(j * 512, 512)], p[:], bias[:])

    def emit_v_proj(st, ts):
        """v projection tiles ts, transposed: Vt[nk, c] (gamma pre-folded)."""
        b = st.b
        if not hasattr(st, "vt") or st.vt is None:
            st.vt = vtpool.tile([128, NT, C], FP8, tag="vt", name=f"vt{b}")
        for t in ts:
            vp = gps.tile([128, 512], F32, tag="g", name=f"vp{b}_{t}")
            for kk in range(0, CC, 2):
                nc.tensor.matmul(vp[:], st.audio[:, kk:kk + 2, ds(t * 128, 128)],
                                 wv_sb[:, kk:kk + 2, :],
                                 start=(kk == 0), stop=(kk == CC - 2),
                                 perf_mode=DR)
            nc.vector.tensor_scalar_mul(st.vt[:, t, :], vp[:], 1.0)

    def emit_energy_pair(st, t):
        """Energy tiles (t, t+1) + exp; row-packed pairs (K=64 each) run
        concurrently in disjoint halves of the PE array."""
        b = st.b
        if not hasattr(st, "pt") or st.pt is None:
            st.pt = ptpool.tile([128, NT, NJ, 512], FP8, tag="pt", name=f"pt{b}")
        ep = [eps.tile([128, NJ, 512], F32, tag="e", name=f"ep{b}_{t+h}")
              for h in range(2)]
        for j in range(NJ):
            for h in range(2):  # h=0 -> rows 0:64, h=1 -> rows 64:128
                hs = ds(h * 64, 64)
                nc.tensor.matmul(ep[h][:, j, :], st.k[hs, ds((t + h) * 128, 128)],
                                 st.q[hs, ds(j * 512, 512)], start=True, stop=True,
                                 tile_position=(h * 64, 0))
        for h in range(2):
            # PT = exp(ET/sqrt(64)); softmax shift-invariance => no max pass
            nc.scalar.activation(st.pt[:, t + h], ep[h][:], EXP, scale=0.125)

    def emit_sums(st):
        """Softmax denominators, pre-broadcast: S[p, nq] = sum_nk PT."""
        b = st.b
        st.sp = [gps.tile([128, 512], F32, tag="g", name=f"sp{b}_{j}")
                 for j in range(NJ)]
        for t in range(0, NT, 2):
            for j in range(NJ):
                nc.tensor.matmul(st.sp[j][:], ones_mat[:], st.pt[:, t:t + 2, j],
                                 start=(t == 0), stop=(t == NT - 2), perf_mode=DR)

    def emit_recip(st):
        b = st.b
        st.recip = misc.tile([128, N], F32, tag="recip", name=f"recip{b}")
        for j in range(NJ):
            nc.vector.reciprocal_approx_fast(st.recip[:, ds(j * 512, 512)],
                                             st.sp[j][:])

    def emit_pv_cc(st, cc, last_batch):
        """PV + residual for one c-chunk:
        out[c, nq] = (gamma*O)/S + (face + gamma*bv)."""
        b = st.b
        op = [gps.tile([128, 512], F32, tag="g", name=f"op{b}_{cc}_{j}")
              for j in range(NJ)]
        for t in range(0, NT, 2):
            for j in range(NJ):
                nc.tensor.matmul(op[j][:], st.vt[:, t:t + 2, ds(cc * 128, 128)],
                                 st.pt[:, t:t + 2, j],
                                 start=(t == 0), stop=(t == NT - 2), perf_mode=DR)
        tmp = tmppool.tile([128, N], BF16, tag="tmp", name=f"tmp{b}_{cc}")
        for j in range(NJ):
            nc.vector.tensor_mul(tmp[:, ds(j * 512, 512)], op[j][:],
                                 st.recip[:, ds(j * 512, 512)])
        fslice = st.facer[:, cc, :]
        if last_batch:
            # drain phase: fine-grained per-half adds so the post-last-MM
            # chain is just one mul+add+128KB DMA; adds alternate
            # VectorE/GpSimd, out halves alternate the sync/gpsimd queues
            for j in range(NJ):
                chunk = cc * NJ + j
                add_eng = nc.vector if chunk % 2 == 0 else nc.gpsimd
                add_eng.tensor_add(fslice[:, ds(j * 512, 512)],
                                   tmp[:, ds(j * 512, 512)],
                                   fslice[:, ds(j * 512, 512)])
                dma_eng = nc.sync if j == 0 else nc.gpsimd
                dma_eng.dma_start(out[b, cc, :, ds(j * 512, 512)],
                                  st.facer[:, cc, ds(j * 512, 512)])
        else:
            nc.gpsimd.tensor_add(fslice, tmp[:], fslice)
            nc.sync.dma_start(out[b, cc, :, ds(0, 512)],
                              st.facer[:, cc, ds(0, 512)])
            nc.gpsimd.dma_start(out[b, cc, :, ds(512, 512)],
                                st.facer[:, cc, ds(512, 512)])

    # ---------------- pipelined emission ----------------
    st = _emit_dma_in(nc, inpool, io, 0)
    _emit_dma_facer(nc, inpool, io, st)
    prev = None
    for b in range(BPC):
        nxt = _emit_dma_in(nc, inpool, io, b + 1) if b + 1 < BPC else None
        st.vt = None
        st.pt = None
        emit_qk_proj(st)
        if prev is None:
            # batch 0: no PV to interleave; spread energy pairs with v-proj
            # so the 2-slot exp PSUM pool never stalls the PE
            emit_energy_pair(st, 0)
            emit_v_proj(st, range(0, 4))
            emit_energy_pair(st, 2)
            emit_v_proj(st, range(4, 8))
            emit_energy_pair(st, 4)
            emit_energy_pair(st, 6)
        else:
            emit_v_proj(st, range(0, 8))
            emit_sums(prev)
            emit_recip(prev)
            emit_energy_pair(st, 0)
            emit_pv_cc(prev, 0, False)
            emit_energy_pair(st, 2)
            emit_pv_cc(prev, 1, False)
            emit_energy_pair(st, 4)
            emit_pv_cc(prev, 2, False)
            emit_energy_pair(st, 6)
            emit_pv_cc(prev, 3, False)
        # bf16 residual prefetch for the next batch, emitted AFTER this
        # iteration's out-DMAs so sync-queue order matches slot-free order
        if nxt is not None:
            _emit_dma_facer(nc, inpool, io, nxt)
        prev, st = st, nxt

    # drain: B-phase of the last batch
    emit_sums(prev)
    emit_recip(prev)
    for cc in range(CC):
        emit_pv_cc(prev, cc, True)


def _build_program():
    global _PROGRAM
    if _PROGRAM is not None:
        return _PROGRAM
    nc = bacc.Bacc("TRN2", target_bir_lowering=False, debug=False,
                   num_devices=N_CORES)
    d = {}
    d["face8"] = nc.dram_tensor("face8", [BPC, CC, 128, N], FP8, kind="ExternalInput").ap()
    d["audio8"] = nc.dram_tensor("audio8", [BPC, CC, 128, N], FP8, kind="ExternalInput").ap()
    d["faceres"] = nc.dram_tensor("faceres", [BPC, CC, 128, N], BF16, kind="ExternalInput").ap()
    d["wqk"] = nc.dram_tensor("wqk", [128, CC, 256], FP8, kind="ExternalInput").ap()
    d["wv"] = nc.dram_tensor("wv", [128, CC, C], FP8, kind="ExternalInput").ap()
    d["bq"] = nc.dram_tensor("bq", [128, 1], F32, kind="ExternalInput").ap()
    d["bk"] = nc.dram_tensor("bk", [128, 1], F32, kind="ExternalInput").ap()
    d["out"] = nc.dram_tensor("out", [BPC, CC, 128, N], BF16, kind="ExternalOutput").ap()

    with tile.TileContext(nc) as tc:
        with ExitStack() as ctx:
            _emit(nc, tc, ctx, d)
    nc.compile()
    _PROGRAM = nc
    return nc


def _make_in_maps(face_feat, audio_feat, Wq, bq, Wk, bk, Wv, bv, gamma):
    fp8 = ml_dtypes.float8_e4m3fn
    bf16 = ml_dtypes.bfloat16
    g = np.float32(np.asarray(gamma).reshape(-1)[0])

    face = np.ascontiguousarray(face_feat.reshape(B, C, N), dtype=np.float32)
    audio = np.ascontiguousarray(audio_feat.reshape(B, C, N), dtype=np.float32)

    # residual folds in gamma*bv (v-bias passes through softmax exactly)
    faceres = (face + (g * bv.astype(np.float32))[None, :, None])
    faceres = faceres.astype(bf16).reshape(B, CC, 128, N)

    face8 = face.astype(fp8).reshape(B, CC, 128, N)
    audio8 = audio.astype(fp8).reshape(B, CC, 128, N)

    def chunk_t(wT):  # [C, M] -> [128, CC, M]
        return np.ascontiguousarray(wT.reshape(CC, 128, -1).transpose(1, 0, 2))

    # q/k weights duplicated along M so projections emit both partition
    # halves (feeds the row-packed energy matmuls); gamma folded into Wv;
    # all three packed into one tensor for a single weights DMA
    wqT = chunk_t(np.concatenate([Wq.T, Wq.T], axis=1).astype(np.float32).astype(fp8))
    wkT = chunk_t(np.concatenate([Wk.T, Wk.T], axis=1).astype(np.float32).astype(fp8))
    wvT = np.ascontiguousarray(chunk_t((g * Wv.astype(np.float32)).T.astype(fp8)))
    wqk = np.ascontiguousarray(np.concatenate([wqT, wkT], axis=2))
    bq2 = np.tile(bq.astype(np.float32).reshape(CQK, 1), (2, 1))
    bk2 = np.tile(bk.astype(np.float32).reshape(CQK, 1), (2, 1))

    in_maps = []
    for i in range(N_CORES):
        sl = slice(i * BPC, (i + 1) * BPC)
        in_maps.append({
            "face8": face8[sl], "audio8": audio8[sl], "faceres": faceres[sl],
            "wqk": wqk, "wv": wvT, "bq": bq2, "bk": bk2,
        })
    return in_maps


def kernel(face_feat, audio_feat, Wq, bq, Wk, bk, Wv, bv, gamma):
    nc = _build_program()
    in_maps = _make_in_maps(face_feat, audio_feat, Wq, bq, Wk, bk, Wv, bv, gamma)
    res = run_bass_kernel_spmd(nc, in_maps, core_ids=list(range(N_CORES)))
    out = np.concatenate([res.results[i]["out"] for i in range(N_CORES)], axis=0)
    return out.astype(np.float32).reshape(B, C, H, W)


# revision 26
# speedup vs baseline: 1.0474x; 1.0474x over previous
"""Trainium2 Bass kernel for CrossModalAttention2d.

Reference computation (per batch element b):
    q = Wq @ face[b] + bq          # [64, 1024]   (face as [C=512, N=1024])
    k = Wk @ audio[b] + bk         # [64, 1024]
    v = Wv @ audio[b] + bv         # [512, 1024]
    attn = softmax(q^T k / 8, axis=-1)          # [1024, 1024]
    out = gamma * (v @ attn^T) + face[b]        # [512, 1024]

Distribution: data-parallel over batch B=32 across 8 NeuronCores
(4 batch elements per core); every core holds the full (small) weights.

Device-side design notes (v2 — software-pipelined):
- All heavy matmuls run in fp8 DoubleRow on TensorE; energy in bf16
  (K=64 row-packed pairs run concurrently in disjoint PE row halves).
- Energy is computed directly in TRANSPOSED layout ET[nk, nq] = k^T q,
  so the attention matrix is produced with nk on partitions — exactly
  the layout the PV matmul needs as its moving operand.
- softmax normalization: max-subtraction + clip(+-50) are numerical
  no-ops for this operator (energies are O(1)); exp(e/8) directly on
  ScalarE, normalize by column sums (ones-matmul + fast reciprocal).
- gamma is folded into Wv and bv ON HOST: Wv_scaled = gamma*Wv, and the
  residual input is face + gamma*bv in bf16 (v-bias passes through
  softmax exactly since attn rows sum to 1). The Vt PSUM->SBUF cast is
  a pure copy, split across ScalarE/VectorE.
- IO is slimmed: residual face in bf16 (not fp32), output in bf16
  (host upcasts) — halves the dominant DMA traffic.
- exp runs as FD=1024 activations over 2-bank PSUM tiles (halves the
  per-instruction overhead on ScalarE, the co-bottleneck engine).
- Residual adds run on the otherwise-idle GpSimd engine (except the
  last batch, where VectorE is used to minimize the serial tail).
- Software pipelining: batch b's energy matmuls are interleaved with
  batch b-1's PV matmuls in the emission (= priority) order, so the
  PE never waits on ScalarE's exp chain and the HAM clock stays warm.
"""

from contextlib import ExitStack

import ml_dtypes
import numpy as np

import concourse.bass as bass
import concourse.mybir as mybir
import concourse.tile as tile
from concourse import bacc
from concourse.bass import ds
from concourse.bass_utils import run_bass_kernel_spmd

N_CORES = 8
B = 32
C = 512
CQK = 64
N = 1024          # Nq = Nk = 32*32
H = W = 32
BPC = B // N_CORES  # batches per core
CC = C // 128       # 4 c-chunks
NT = N // 128       # 8 nk-tiles
NJ = N // 512       # 2 nq halves (PSUM bank = 512 fp32)

BF16 = mybir.dt.bfloat16
FP8 = mybir.dt.float8e4
F32 = mybir.dt.float32
DR = mybir.MatmulPerfMode.DoubleRow
EXP = mybir.ActivationFunctionType.Exp

_PROGRAM = None


class _BatchState:
    """SBUF tiles of one in-flight batch."""
    __slots__ = ("b", "face", "audio", "facer", "q", "k", "vt", "pt",
                 "recip", "sp")


def _emit_dma_in(nc, inpool, io, b):
    """Issue face/audio input DMAs for batch b (fp8 projection inputs).
    The bf16 residual input is DMAed separately (see _emit_dma_facer) so
    its slot-reuse wait can never sit ahead of the out-DMAs that free it
    in the in-order sync queue."""
    face8, audio8 = io["face8"], io["audio8"]
    st = _BatchState()
    st.b = b
    st.face = inpool.tile([128, CC, N], FP8, tag="face", name=f"face{b}")
    st.audio = inpool.tile([128, CC, N], FP8, tag="audio", name=f"audio{b}")
    # full-width rows (contiguous 1 KiB lines -> full HBM efficiency);
    # ScalarE carries no DMA descriptors at all: it is the exp engine and
    # must never be the resource the PE waits on
    for kk in range(CC):
        nc.sync.dma_start(st.face[:, kk, :], face8[b, kk])
        nc.sync.dma_start(st.audio[:, kk, :], audio8[b, kk])
    return st


def _emit_dma_facer(nc, inpool, io, st):
    # gpsimd queue: keeps the bf16 residual stream off the sync queue
    st.facer = inpool.tile([128, CC, N], BF16, tag="facer", name=f"facer{st.b}")
    for kk in range(CC):
        nc.gpsimd.dma_start(st.facer[:, kk, :], io["faceres"][st.b, kk])


def _emit(nc, tc, ctx, io):
    wpool = ctx.enter_context(tc.tile_pool(name="weights", bufs=1))
    inpool = ctx.enter_context(tc.tile_pool(name="inputs", bufs=2))
    qkpool = ctx.enter_context(tc.tile_pool(name="qk", bufs=2))
    vtpool = ctx.enter_context(tc.tile_pool(name="vt", bufs=2))
    ptpool = ctx.enter_context(tc.tile_pool(name="pt", bufs=2))
    misc = ctx.enter_context(tc.tile_pool(name="misc", bufs=2))
    tmppool = ctx.enter_context(tc.tile_pool(name="tmp", bufs=4))
    gps = ctx.enter_context(tc.tile_pool(name="gps", bufs=4, space="PSUM"))
    eps = ctx.enter_context(tc.tile_pool(name="eps", bufs=2, space="PSUM"))

    # --- persistent weights/constants ---
    # memsets first: the PE warm-up matmuls depend only on these
    ones_mat = wpool.tile([128, 2, 128], FP8)
    nc.vector.memset(ones_mat[:], 1.0)
    warm_rhs = wpool.tile([128, 2, 512], FP8)
    nc.vector.memset(warm_rhs[:], 1.0)
    warm_sb = wpool.tile([128, 1], F32)
    nc.vector.memset(warm_sb[:], 0.0)

    # ~3.5us of dummy matmuls on memset data: keeps the PE busy through
    # the HAM activity window during the initial input-DMA wait, so the
    # first real matmuls run at 2.4 GHz instead of the cold 1.2 GHz
    warm_mm = gps.tile([128, 512], F32, tag="g")
    for _ in range(8):
        nc.tensor.matmul(warm_mm[:], ones_mat[:], warm_rhs[:],
                         start=True, stop=True, perf_mode=DR)

    # warm the ScalarE exp table off the critical path
    warm_ps = gps.tile([128, 1], F32, tag="g")
    nc.scalar.activation(warm_ps[:], warm_sb[:], EXP)

    # weights on the gpsimd HW queue (parallel with face/audio streams);
    # wqk (needed by the first matmuls) is a separate small DMA so the
    # pipeline can start before wv lands
    wqk_sb = wpool.tile([128, CC, 256], FP8)
    nc.gpsimd.dma_start(wqk_sb[:], io["wqk"][:])
    wv_sb = wpool.tile([128, CC, C], FP8)  # pre-scaled by gamma on host
    nc.gpsimd.dma_start(wv_sb[:], io["wv"][:])
    WQ_OFF, WK_OFF = 0, 128
    bq_sb = wpool.tile([128, 1], F32)
    nc.gpsimd.dma_start(bq_sb[:], io["bq"][:])
    bk_sb = wpool.tile([128, 1], F32)
    nc.gpsimd.dma_start(bk_sb[:], io["bk"][:])

    out = io["out"]

    def emit_qk_proj(st):
        """q/k projections: [128, 1024] (dup halves) = [W|W] @ x."""
        b = st.b
        st.q = qkpool.tile([128, N], BF16, tag="q", name=f"q{b}")
        st.k = qkpool.tile([128, N], BF16, tag="k", name=f"k{b}")
        for (w_off, x, dst, bias) in ((WQ_OFF, st.face, st.q, bq_sb),
                                      (WK_OFF, st.audio, st.k, bk_sb)):
            for j in range(NJ):
                p = gps.tile([128, 512], F32, tag="g", name=f"qkp{b}_{j}")
                for kk in range(0, CC, 2):
                    nc.tensor.matmul(p[:], wqk_sb[:, kk:kk + 2, ds(w_off, 128)],
                                     x[:, kk:kk + 2, ds(j * 512, 512)],
                                     start=(kk == 0), stop=(kk == CC - 2),
                                     perf_mode=DR)
                nc.vector.tensor_scalar_add(dst[:, ds(j * 512, 512)], p[:], bias[:])

    def emit_v_proj(st, ts):
        """v projection tiles ts, transposed: Vt[nk, c] (gamma pre-folded)."""
        b = st.b
        if not hasattr(st, "vt") or st.vt is None:
            st.vt = vtpool.tile([128, NT, C], FP8, tag="vt", name=f"vt{b}")
        for t in ts:
            vp = gps.tile([128, 512], F32, tag="g", name=f"vp{b}_{t}")
            for kk in range(0, CC, 2):
                nc.tensor.matmul(vp[:], st.audio[:, kk:kk + 2, ds(t * 128, 128)],
                                 wv_sb[:, kk:kk + 2, :],
                                 start=(kk == 0), stop=(kk == CC - 2),
                                 perf_mode=DR)
            if t % 2 == 0:
                nc.scalar.copy(st.vt[:, t, :], vp[:])
            else:
                nc.vector.tensor_scalar_mul(st.vt[:, t, :], vp[:], 1.0)

    def emit_energy_pair(st, t):
        """Energy tiles (t, t+1) + exp; row-packed pairs (K=64 each) run
        concurrently in disjoint halves of the PE array."""
        b = st.b
        if not hasattr(st, "pt") or st.pt is None:
            st.pt = ptpool.tile([128, NT, NJ, 512], FP8, tag="pt", name=f"pt{b}")
        ep = [eps.tile([128, NJ, 512], F32, tag="e", name=f"ep{b}_{t+h}")
              for h in range(2)]
        for j in range(NJ):
            for h in range(2):  # h=0 -> rows 0:64, h=1 -> rows 64:128
                hs = ds(h * 64, 64)
                nc.tensor.matmul(ep[h][:, j, :], st.k[hs, ds((t + h) * 128, 128)],
                                 st.q[hs, ds(j * 512, 512)], start=True, stop=True,
                                 tile_position=(h * 64, 0))
        for h in range(2):
            # PT = exp(ET/sqrt(64)); softmax shift-invariance => no max pass
            nc.scalar.activation(st.pt[:, t + h], ep[h][:], EXP, scale=0.125)

    def emit_sums(st):
        """Softmax denominators, pre-broadcast: S[p, nq] = sum_nk PT."""
        b = st.b
        st.sp = [gps.tile([128, 512], F32, tag="g", name=f"sp{b}_{j}")
                 for j in range(NJ)]
        for t in range(0, NT, 2):
            for j in range(NJ):
                nc.tensor.matmul(st.sp[j][:], ones_mat[:], st.pt[:, t:t + 2, j],
                                 start=(t == 0), stop=(t == NT - 2), perf_mode=DR)

    def emit_recip(st):
        b = st.b
        st.recip = misc.tile([128, N], F32, tag="recip", name=f"recip{b}")
        for j in range(NJ):
            nc.vector.reciprocal_approx_fast(st.recip[:, ds(j * 512, 512)],
                                             st.sp[j][:])

    def emit_pv_cc(st, cc, last_batch):
        """PV + residual for one c-chunk:
        out[c, nq] = (gamma*O)/S + (face + gamma*bv)."""
        b = st.b
        op = [gps.tile([128, 512], F32, tag="g", name=f"op{b}_{cc}_{j}")
              for j in range(NJ)]
        for t in range(0, NT, 2):
            for j in range(NJ):
                nc.tensor.matmul(op[j][:], st.vt[:, t:t + 2, ds(cc * 128, 128)],
                                 st.pt[:, t:t + 2, j],
                                 start=(t == 0), stop=(t == NT - 2), perf_mode=DR)
        tmp = tmppool.tile([128, N], BF16, tag="tmp", name=f"tmp{b}_{cc}")
        for j in range(NJ):
            nc.vector.tensor_mul(tmp[:, ds(j * 512, 512)], op[j][:],
                                 st.recip[:, ds(j * 512, 512)])
        fslice = st.facer[:, cc, :]
        if last_batch:
            # drain phase: fine-grained per-half adds so the post-last-MM
            # chain is just one mul+add+128KB DMA; adds alternate
            # VectorE/GpSimd, out halves alternate the sync/gpsimd queues
            for j in range(NJ):
                chunk = cc * NJ + j
                add_eng = nc.vector if chunk % 2 == 0 else nc.gpsimd
                add_eng.tensor_add(fslice[:, ds(j * 512, 512)],
                                   tmp[:, ds(j * 512, 512)],
                                   fslice[:, ds(j * 512, 512)])
                dma_eng = nc.sync if j == 0 else nc.gpsimd
                dma_eng.dma_start(out[b, cc, :, ds(j * 512, 512)],
                                  st.facer[:, cc, ds(j * 512, 512)])
        else:
            nc.gpsimd.tensor_add(fslice, tmp[:], fslice)
            nc.sync.dma_start(out[b, cc, :, ds(0, 512)],
                              st.facer[:, cc, ds(0, 512)])
            nc.gpsimd.dma_start(out[b, cc, :, ds(512, 512)],
                                st.facer[:, cc, ds(512, 512)])

    # ---------------- pipelined emission ----------------
    st = _emit_dma_in(nc, inpool, io, 0)
    _emit_dma_facer(nc, inpool, io, st)
    prev = None
    for b in range(BPC):
        nxt = _emit_dma_in(nc, inpool, io, b + 1) if b + 1 < BPC else None
        st.vt = None
        st.pt = None
        emit_qk_proj(st)
        if prev is None:
            # batch 0: no PV to interleave; spread energy pairs with v-proj
            # so the 2-slot exp PSUM pool never stalls the PE
            emit_energy_pair(st, 0)
            emit_v_proj(st, range(0, 4))
            emit_energy_pair(st, 2)
            emit_v_proj(st, range(4, 8))
            emit_energy_pair(st, 4)
            emit_energy_pair(st, 6)
        else:
            emit_v_proj(st, range(0, 8))
            emit_sums(prev)
            emit_recip(prev)
            emit_energy_pair(st, 0)
            emit_pv_cc(prev, 0, False)
            emit_energy_pair(st, 2)
            emit_pv_cc(prev, 1, False)
            emit_energy_pair(st, 4)
            emit_pv_cc(prev, 2, False)
            emit_energy_pair(st, 6)
            emit_pv_cc(prev, 3, False)
        # bf16 residual prefetch for the next batch, emitted AFTER this
        # iteration's out-DMAs so sync-queue order matches slot-free order
        if nxt is not None:
            _emit_dma_facer(nc, inpool, io, nxt)
        prev, st = st, nxt

    # drain: B-phase of the last batch
    emit_sums(prev)
    emit_recip(prev)
    for cc in range(CC):
        emit_pv_cc(prev, cc, True)


def _build_program():
    global _PROGRAM
    if _PROGRAM is not None:
        return _PROGRAM
    nc = bacc.Bacc("TRN2", target_bir_lowering=False, debug=False,
                   num_devices=N_CORES)
    d = {}
    d["face8"] = nc.dram_tensor("face8", [BPC, CC, 128, N], FP8, kind="ExternalInput").ap()
    d["audio8"] = nc.dram_tensor("audio8", [BPC, CC, 128, N], FP8, kind="ExternalInput").ap()
    d["faceres"] = nc.dram_tensor("faceres", [BPC, CC, 128, N], BF16, kind="ExternalInput").ap()
    d["wqk"] = nc.dram_tensor("wqk", [128, CC, 256], FP8, kind="ExternalInput").ap()
    d["wv"] = nc.dram_tensor("wv", [128, CC, C], FP8, kind="ExternalInput").ap()
    d["bq"] = nc.dram_tensor("bq", [128, 1], F32, kind="ExternalInput").ap()
    d["bk"] = nc.dram_tensor("bk", [128, 1], F32, kind="ExternalInput").ap()
    d["out"] = nc.dram_tensor("out", [BPC, CC, 128, N], BF16, kind="ExternalOutput").ap()

    with tile.TileContext(nc) as tc:
        with ExitStack() as ctx:
            _emit(nc, tc, ctx, d)
    nc.compile()
    _PROGRAM = nc
    return nc


def _make_in_maps(face_feat, audio_feat, Wq, bq, Wk, bk, Wv, bv, gamma):
    fp8 = ml_dtypes.float8_e4m3fn
    bf16 = ml_dtypes.bfloat16
    g = np.float32(np.asarray(gamma).reshape(-1)[0])

    face = np.ascontiguousarray(face_feat.reshape(B, C, N), dtype=np.float32)
    audio = np.ascontiguousarray(audio_feat.reshape(B, C, N), dtype=np.float32)

    # residual folds in gamma*bv (v-bias passes through softmax exactly)
    faceres = (face + (g * bv.astype(np.float32))[None, :, None])
    faceres = faceres.astype(bf16).reshape(B, CC, 128, N)

    face8 = face.astype(fp8).reshape(B, CC, 128, N)
    audio8 = audio.astype(fp8).reshape(B, CC, 128, N)

    def chunk_t(wT):  # [C, M] -> [128, CC, M]
        return np.ascontiguousarray(wT.reshape(CC, 128, -1).transpose(1, 0, 2))

    # q/k weights duplicated along M so projections emit both partition
    # halves (feeds the row-packed energy matmuls); gamma folded into Wv;
    # all three packed into one tensor for a single weights DMA
    wqT = chunk_t(np.concatenate([Wq.T, Wq.T], axis=1).astype(np.float32).astype(fp8))
    wkT = chunk_t(np.concatenate([Wk.T, Wk.T], axis=1).astype(np.float32).astype(fp8))
    wvT = np.ascontiguousarray(chunk_t((g * Wv.astype(np.float32)).T.astype(fp8)))
    wqk = np.ascontiguousarray(np.concatenate([wqT, wkT], axis=2))
    bq2 = np.tile(bq.astype(np.float32).reshape(CQK, 1), (2, 1))
    bk2 = np.tile(bk.astype(np.float32).reshape(CQK, 1), (2, 1))

    in_maps = []
    for i in range(N_CORES):
        sl = slice(i * BPC, (i + 1) * BPC)
        in_maps.append({
            "face8": face8[sl], "audio8": audio8[sl], "faceres": faceres[sl],
            "wqk": wqk, "wv": wvT, "bq": bq2, "bk": bk2,
        })
    return in_maps


def kernel(face_feat, audio_feat, Wq, bq, Wk, bk, Wv, bv, gamma):
    nc = _build_program()
    in_maps = _make_in_maps(face_feat, audio_feat, Wq, bq, Wk, bk, Wv, bv, gamma)
    res = run_bass_kernel_spmd(nc, in_maps, core_ids=list(range(N_CORES)))
    out = np.concatenate([res.results[i]["out"] for i in range(N_CORES)], axis=0)
    return out.astype(np.float32).reshape(B, C, H, W)


# revision 27
# speedup vs baseline: 1.0554x; 1.0076x over previous
"""Trainium2 Bass kernel for CrossModalAttention2d.

Reference computation (per batch element b):
    q = Wq @ face[b] + bq          # [64, 1024]   (face as [C=512, N=1024])
    k = Wk @ audio[b] + bk         # [64, 1024]
    v = Wv @ audio[b] + bv         # [512, 1024]
    attn = softmax(q^T k / 8, axis=-1)          # [1024, 1024]
    out = gamma * (v @ attn^T) + face[b]        # [512, 1024]

Distribution: data-parallel over batch B=32 across 8 NeuronCores
(4 batch elements per core); every core holds the full (small) weights.

Device-side design notes (v2 — software-pipelined):
- All heavy matmuls run in fp8 DoubleRow on TensorE; energy in bf16
  (K=64 row-packed pairs run concurrently in disjoint PE row halves).
- Energy is computed directly in TRANSPOSED layout ET[nk, nq] = k^T q,
  so the attention matrix is produced with nk on partitions — exactly
  the layout the PV matmul needs as its moving operand.
- softmax normalization: max-subtraction + clip(+-50) are numerical
  no-ops for this operator (energies are O(1)); exp(e/8) directly on
  ScalarE, normalize by column sums (ones-matmul + fast reciprocal).
- gamma is folded into Wv and bv ON HOST: Wv_scaled = gamma*Wv, and the
  residual input is face + gamma*bv in bf16 (v-bias passes through
  softmax exactly since attn rows sum to 1). The Vt PSUM->SBUF cast is
  a pure copy, split across ScalarE/VectorE.
- IO is slimmed: residual face in bf16 (not fp32), output in bf16
  (host upcasts) — halves the dominant DMA traffic.
- exp runs as FD=1024 activations over 2-bank PSUM tiles (halves the
  per-instruction overhead on ScalarE, the co-bottleneck engine).
- Residual adds run on the otherwise-idle GpSimd engine (except the
  last batch, where VectorE is used to minimize the serial tail).
- Software pipelining: batch b's energy matmuls are interleaved with
  batch b-1's PV matmuls in the emission (= priority) order, so the
  PE never waits on ScalarE's exp chain and the HAM clock stays warm.
"""

from contextlib import ExitStack

import ml_dtypes
import numpy as np

import concourse.bass as bass
import concourse.mybir as mybir
import concourse.tile as tile
from concourse import bacc
from concourse.bass import ds
from concourse.bass_utils import run_bass_kernel_spmd

N_CORES = 8
B = 32
C = 512
CQK = 64
N = 1024          # Nq = Nk = 32*32
H = W = 32
BPC = B // N_CORES  # batches per core
CC = C // 128       # 4 c-chunks
NT = N // 128       # 8 nk-tiles
NJ = N // 512       # 2 nq halves (PSUM bank = 512 fp32)

BF16 = mybir.dt.bfloat16
FP8 = mybir.dt.float8e4
F32 = mybir.dt.float32
DR = mybir.MatmulPerfMode.DoubleRow
EXP = mybir.ActivationFunctionType.Exp

_PROGRAM = None


class _BatchState:
    """SBUF tiles of one in-flight batch."""
    __slots__ = ("b", "face", "audio", "facer", "q", "k", "vt", "pt",
                 "recip", "sp")


def _emit_dma_in(nc, inpool, io, b):
    """Issue face/audio input DMAs for batch b (fp8 projection inputs).
    The bf16 residual input is DMAed separately (see _emit_dma_facer) so
    its slot-reuse wait can never sit ahead of the out-DMAs that free it
    in the in-order sync queue."""
    face8, audio8 = io["face8"], io["audio8"]
    st = _BatchState()
    st.b = b
    st.face = inpool.tile([128, CC, N], FP8, tag="face", name=f"face{b}")
    st.audio = inpool.tile([128, CC, N], FP8, tag="audio", name=f"audio{b}")
    # full-width rows (contiguous 1 KiB lines -> full HBM efficiency);
    # ScalarE carries no DMA descriptors at all: it is the exp engine and
    # must never be the resource the PE waits on
    for kk in range(CC):
        nc.sync.dma_start(st.face[:, kk, :], face8[b, kk])
        nc.sync.dma_start(st.audio[:, kk, :], audio8[b, kk])
    return st


def _emit_dma_facer(nc, inpool, io, st):
    # gpsimd queue: keeps the bf16 residual stream off the sync queue
    st.facer = inpool.tile([128, CC, N], BF16, tag="facer", name=f"facer{st.b}")
    for kk in range(CC):
        nc.gpsimd.dma_start(st.facer[:, kk, :], io["faceres"][st.b, kk])


def _emit(nc, tc, ctx, io):
    wpool = ctx.enter_context(tc.tile_pool(name="weights", bufs=1))
    inpool = ctx.enter_context(tc.tile_pool(name="inputs", bufs=2))
    qkpool = ctx.enter_context(tc.tile_pool(name="qk", bufs=2))
    vtpool = ctx.enter_context(tc.tile_pool(name="vt", bufs=2))
    ptpool = ctx.enter_context(tc.tile_pool(name="pt", bufs=2))
    misc = ctx.enter_context(tc.tile_pool(name="misc", bufs=2))
    tmppool = ctx.enter_context(tc.tile_pool(name="tmp", bufs=4))
    gps = ctx.enter_context(tc.tile_pool(name="gps", bufs=4, space="PSUM"))
    eps = ctx.enter_context(tc.tile_pool(name="eps", bufs=2, space="PSUM"))

    # --- persistent weights/constants ---
    # all fp8 weights packed into one DMA ([wq | wk | wv] along free dim);
    # on the sync queue so nothing serializes behind the exp table load
    wqkv_sb = wpool.tile([128, CC, 128 + 128 + C], FP8)
    nc.sync.dma_start(wqkv_sb[:], io["wqkv"][:])
    # free-dim offsets of the packed [wq | wk | wv(gamma-scaled)] weights
    WQ_OFF, WK_OFF, WV_OFF = 0, 128, 256
    bq_sb = wpool.tile([128, 1], F32)
    nc.sync.dma_start(bq_sb[:], io["bq"][:])
    bk_sb = wpool.tile([128, 1], F32)
    nc.sync.dma_start(bk_sb[:], io["bk"][:])
    ones_mat = wpool.tile([128, 2, 128], FP8)
    nc.vector.memset(ones_mat[:], 1.0)

    # warm the ScalarE exp table off the critical path
    warm_ps = gps.tile([128, 1], F32, tag="g")
    warm_sb = wpool.tile([128, 1], F32)
    nc.vector.memset(warm_sb[:], 0.0)
    nc.scalar.activation(warm_ps[:], warm_sb[:], EXP)

    out = io["out"]

    def emit_qk_proj(st):
        """q/k projections: [128, 1024] (dup halves) = [W|W] @ x."""
        b = st.b
        st.q = qkpool.tile([128, N], BF16, tag="q", name=f"q{b}")
        st.k = qkpool.tile([128, N], BF16, tag="k", name=f"k{b}")
        for (w_off, x, dst, bias) in ((WQ_OFF, st.face, st.q, bq_sb),
                                      (WK_OFF, st.audio, st.k, bk_sb)):
            for j in range(NJ):
                p = gps.tile([128, 512], F32, tag="g", name=f"qkp{b}_{j}")
                for kk in range(0, CC, 2):
                    nc.tensor.matmul(p[:], wqkv_sb[:, kk:kk + 2, ds(w_off, 128)],
                                     x[:, kk:kk + 2, ds(j * 512, 512)],
                                     start=(kk == 0), stop=(kk == CC - 2),
                                     perf_mode=DR)
                nc.vector.tensor_scalar_add(dst[:, ds(j * 512, 512)], p[:], bias[:])

    def emit_v_proj(st, ts):
        """v projection tiles ts, transposed: Vt[nk, c] (gamma pre-folded)."""
        b = st.b
        if not hasattr(st, "vt") or st.vt is None:
            st.vt = vtpool.tile([128, NT, C], FP8, tag="vt", name=f"vt{b}")
        for t in ts:
            vp = gps.tile([128, 512], F32, tag="g", name=f"vp{b}_{t}")
            for kk in range(0, CC, 2):
                nc.tensor.matmul(vp[:], st.audio[:, kk:kk + 2, ds(t * 128, 128)],
                                 wqkv_sb[:, kk:kk + 2, ds(WV_OFF, C)],
                                 start=(kk == 0), stop=(kk == CC - 2),
                                 perf_mode=DR)
            if t % 2 == 0:
                nc.scalar.copy(st.vt[:, t, :], vp[:])
            else:
                nc.vector.tensor_scalar_mul(st.vt[:, t, :], vp[:], 1.0)

    def emit_energy_pair(st, t):
        """Energy tiles (t, t+1) + exp; row-packed pairs (K=64 each) run
        concurrently in disjoint halves of the PE array."""
        b = st.b
        if not hasattr(st, "pt") or st.pt is None:
            st.pt = ptpool.tile([128, NT, NJ, 512], FP8, tag="pt", name=f"pt{b}")
        ep = [eps.tile([128, NJ, 512], F32, tag="e", name=f"ep{b}_{t+h}")
              for h in range(2)]
        for j in range(NJ):
            for h in range(2):  # h=0 -> rows 0:64, h=1 -> rows 64:128
                hs = ds(h * 64, 64)
                nc.tensor.matmul(ep[h][:, j, :], st.k[hs, ds((t + h) * 128, 128)],
                                 st.q[hs, ds(j * 512, 512)], start=True, stop=True,
                                 tile_position=(h * 64, 0))
        for h in range(2):
            # PT = exp(ET/sqrt(64)); softmax shift-invariance => no max pass
            nc.scalar.activation(st.pt[:, t + h], ep[h][:], EXP, scale=0.125)

    def emit_sums(st):
        """Softmax denominators, pre-broadcast: S[p, nq] = sum_nk PT."""
        b = st.b
        st.sp = [gps.tile([128, 512], F32, tag="g", name=f"sp{b}_{j}")
                 for j in range(NJ)]
        for t in range(0, NT, 2):
            for j in range(NJ):
                nc.tensor.matmul(st.sp[j][:], ones_mat[:], st.pt[:, t:t + 2, j],
                                 start=(t == 0), stop=(t == NT - 2), perf_mode=DR)

    def emit_recip(st):
        b = st.b
        st.recip = misc.tile([128, N], F32, tag="recip", name=f"recip{b}")
        for j in range(NJ):
            nc.vector.reciprocal_approx_fast(st.recip[:, ds(j * 512, 512)],
                                             st.sp[j][:])

    def emit_pv_cc(st, cc, last_batch):
        """PV + residual for one c-chunk:
        out[c, nq] = (gamma*O)/S + (face + gamma*bv)."""
        b = st.b
        op = [gps.tile([128, 512], F32, tag="g", name=f"op{b}_{cc}_{j}")
              for j in range(NJ)]
        for t in range(0, NT, 2):
            for j in range(NJ):
                nc.tensor.matmul(op[j][:], st.vt[:, t:t + 2, ds(cc * 128, 128)],
                                 st.pt[:, t:t + 2, j],
                                 start=(t == 0), stop=(t == NT - 2), perf_mode=DR)
        tmp = tmppool.tile([128, N], BF16, tag="tmp", name=f"tmp{b}_{cc}")
        for j in range(NJ):
            nc.vector.tensor_mul(tmp[:, ds(j * 512, 512)], op[j][:],
                                 st.recip[:, ds(j * 512, 512)])
        fslice = st.facer[:, cc, :]
        if last_batch:
            # VectorE per-half adds: minimal serial tail after the last MM
            for j in range(NJ):
                nc.vector.tensor_add(fslice[:, ds(j * 512, 512)],
                                     tmp[:, ds(j * 512, 512)],
                                     fslice[:, ds(j * 512, 512)])
                nc.sync.dma_start(out[b, cc, :, ds(j * 512, 512)],
                                  st.facer[:, cc, ds(j * 512, 512)])
        else:
            nc.gpsimd.tensor_add(fslice, tmp[:], fslice)
            nc.sync.dma_start(out[b, cc], fslice)

    # ---------------- pipelined emission ----------------
    st = _emit_dma_in(nc, inpool, io, 0)
    _emit_dma_facer(nc, inpool, io, st)
    prev = None
    for b in range(BPC):
        nxt = _emit_dma_in(nc, inpool, io, b + 1) if b + 1 < BPC else None
        st.vt = None
        st.pt = None
        emit_qk_proj(st)
        if prev is None:
            # batch 0: no PV to interleave; spread energy pairs with v-proj
            # so the 2-slot exp PSUM pool never stalls the PE
            emit_energy_pair(st, 0)
            emit_v_proj(st, range(0, 4))
            emit_energy_pair(st, 2)
            emit_v_proj(st, range(4, 8))
            emit_energy_pair(st, 4)
            emit_energy_pair(st, 6)
        else:
            emit_v_proj(st, range(0, 8))
            emit_sums(prev)
            emit_recip(prev)
            emit_energy_pair(st, 0)
            emit_pv_cc(prev, 0, False)
            emit_energy_pair(st, 2)
            emit_pv_cc(prev, 1, False)
            emit_energy_pair(st, 4)
            emit_pv_cc(prev, 2, False)
            emit_energy_pair(st, 6)
            emit_pv_cc(prev, 3, False)
        # bf16 residual prefetch for the next batch, emitted AFTER this
        # iteration's out-DMAs so sync-queue order matches slot-free order
        if nxt is not None:
            _emit_dma_facer(nc, inpool, io, nxt)
        prev, st = st, nxt

    # drain: B-phase of the last batch
    emit_sums(prev)
    emit_recip(prev)
    for cc in range(CC):
        emit_pv_cc(prev, cc, True)


def _build_program():
    global _PROGRAM
    if _PROGRAM is not None:
        return _PROGRAM
    nc = bacc.Bacc("TRN2", target_bir_lowering=False, debug=False,
                   num_devices=N_CORES)
    d = {}
    d["face8"] = nc.dram_tensor("face8", [BPC, CC, 128, N], FP8, kind="ExternalInput").ap()
    d["audio8"] = nc.dram_tensor("audio8", [BPC, CC, 128, N], FP8, kind="ExternalInput").ap()
    d["faceres"] = nc.dram_tensor("faceres", [BPC, CC, 128, N], BF16, kind="ExternalInput").ap()
    d["wqkv"] = nc.dram_tensor("wqkv", [128, CC, 128 + 128 + C], FP8, kind="ExternalInput").ap()
    d["bq"] = nc.dram_tensor("bq", [128, 1], F32, kind="ExternalInput").ap()
    d["bk"] = nc.dram_tensor("bk", [128, 1], F32, kind="ExternalInput").ap()
    d["out"] = nc.dram_tensor("out", [BPC, CC, 128, N], BF16, kind="ExternalOutput").ap()

    with tile.TileContext(nc) as tc:
        with ExitStack() as ctx:
            _emit(nc, tc, ctx, d)
    nc.compile()
    _PROGRAM = nc
    return nc


def _make_in_maps(face_feat, audio_feat, Wq, bq, Wk, bk, Wv, bv, gamma):
    fp8 = ml_dtypes.float8_e4m3fn
    bf16 = ml_dtypes.bfloat16
    g = np.float32(np.asarray(gamma).reshape(-1)[0])

    face = np.ascontiguousarray(face_feat.reshape(B, C, N), dtype=np.float32)
    audio = np.ascontiguousarray(audio_feat.reshape(B, C, N), dtype=np.float32)

    # residual folds in gamma*bv (v-bias passes through softmax exactly)
    faceres = (face + (g * bv.astype(np.float32))[None, :, None])
    faceres = faceres.astype(bf16).reshape(B, CC, 128, N)

    face8 = face.astype(fp8).reshape(B, CC, 128, N)
    audio8 = audio.astype(fp8).reshape(B, CC, 128, N)

    def chunk_t(wT):  # [C, M] -> [128, CC, M]
        return np.ascontiguousarray(wT.reshape(CC, 128, -1).transpose(1, 0, 2))

    # q/k weights duplicated along M so projections emit both partition
    # halves (feeds the row-packed energy matmuls); gamma folded into Wv;
    # all three packed into one tensor for a single weights DMA
    wqT = chunk_t(np.concatenate([Wq.T, Wq.T], axis=1).astype(np.float32).astype(fp8))
    wkT = chunk_t(np.concatenate([Wk.T, Wk.T], axis=1).astype(np.float32).astype(fp8))
    wvT = chunk_t((g * Wv.astype(np.float32)).T.astype(fp8))
    wqkv = np.ascontiguousarray(np.concatenate([wqT, wkT, wvT], axis=2))
    bq2 = np.tile(bq.astype(np.float32).reshape(CQK, 1), (2, 1))
    bk2 = np.tile(bk.astype(np.float32).reshape(CQK, 1), (2, 1))

    in_maps = []
    for i in range(N_CORES):
        sl = slice(i * BPC, (i + 1) * BPC)
        in_maps.append({
            "face8": face8[sl], "audio8": audio8[sl], "faceres": faceres[sl],
            "wqkv": wqkv, "bq": bq2, "bk": bk2,
        })
    return in_maps


def kernel(face_feat, audio_feat, Wq, bq, Wk, bk, Wv, bv, gamma):
    nc = _build_program()
    in_maps = _make_in_maps(face_feat, audio_feat, Wq, bq, Wk, bk, Wv, bv, gamma)
    res = run_bass_kernel_spmd(nc, in_maps, core_ids=list(range(N_CORES)))
    out = np.concatenate([res.results[i]["out"] for i in range(N_CORES)], axis=0)
    return out.astype(np.float32).reshape(B, C, H, W)


# revision 28
# speedup vs baseline: 1.0680x; 1.0120x over previous
"""Trainium2 Bass kernel for CrossModalAttention2d.

Reference computation (per batch element b):
    q = Wq @ face[b] + bq          # [64, 1024]   (face as [C=512, N=1024])
    k = Wk @ audio[b] + bk         # [64, 1024]
    v = Wv @ audio[b] + bv         # [512, 1024]
    attn = softmax(q^T k / 8, axis=-1)          # [1024, 1024]
    out = gamma * (v @ attn^T) + face[b]        # [512, 1024]

Distribution: data-parallel over batch B=32 across 8 NeuronCores
(4 batch elements per core); every core holds the full (small) weights.

Device-side design notes (v2 — software-pipelined):
- All heavy matmuls run in fp8 DoubleRow on TensorE; energy in bf16
  (K=64 row-packed pairs run concurrently in disjoint PE row halves).
- Energy is computed directly in TRANSPOSED layout ET[nk, nq] = k^T q,
  so the attention matrix is produced with nk on partitions — exactly
  the layout the PV matmul needs as its moving operand.
- softmax normalization: max-subtraction + clip(+-50) are numerical
  no-ops for this operator (energies are O(1)); exp(e/8) directly on
  ScalarE, normalize by column sums (ones-matmul + fast reciprocal).
- gamma is folded into Wv and bv ON HOST: Wv_scaled = gamma*Wv, and the
  residual input is face + gamma*bv in bf16 (v-bias passes through
  softmax exactly since attn rows sum to 1). The Vt PSUM->SBUF cast is
  a pure copy, split across ScalarE/VectorE.
- IO is slimmed: residual face in bf16 (not fp32), output in bf16
  (host upcasts) — halves the dominant DMA traffic.
- exp runs as FD=1024 activations over 2-bank PSUM tiles (halves the
  per-instruction overhead on ScalarE, the co-bottleneck engine).
- Residual adds run on the otherwise-idle GpSimd engine (except the
  last batch, where VectorE is used to minimize the serial tail).
- Software pipelining: batch b's energy matmuls are interleaved with
  batch b-1's PV matmuls in the emission (= priority) order, so the
  PE never waits on ScalarE's exp chain and the HAM clock stays warm.
"""

from contextlib import ExitStack

import ml_dtypes
import numpy as np

import concourse.bass as bass
import concourse.mybir as mybir
import concourse.tile as tile
from concourse import bacc
from concourse.bass import ds
from concourse.bass_utils import run_bass_kernel_spmd

N_CORES = 8
B = 32
C = 512
CQK = 64
N = 1024          # Nq = Nk = 32*32
H = W = 32
BPC = B // N_CORES  # batches per core
CC = C // 128       # 4 c-chunks
NT = N // 128       # 8 nk-tiles
NJ = N // 512       # 2 nq halves (PSUM bank = 512 fp32)

BF16 = mybir.dt.bfloat16
FP8 = mybir.dt.float8e4
F32 = mybir.dt.float32
DR = mybir.MatmulPerfMode.DoubleRow
EXP = mybir.ActivationFunctionType.Exp

_PROGRAM = None


class _BatchState:
    """SBUF tiles of one in-flight batch."""
    __slots__ = ("b", "face", "audio", "facer", "q", "k", "vt", "pt",
                 "recip", "sp")


def _emit_dma_in(nc, inpool, io, b):
    """Issue face/audio input DMAs for batch b (fp8 projection inputs).
    The bf16 residual input is DMAed separately (see _emit_dma_facer) so
    its slot-reuse wait can never sit ahead of the out-DMAs that free it
    in the in-order sync queue."""
    face8, audio8 = io["face8"], io["audio8"]
    st = _BatchState()
    st.b = b
    st.face = inpool.tile([128, CC, N], FP8, tag="face", name=f"face{b}")
    st.audio = inpool.tile([128, CC, N], FP8, tag="audio", name=f"audio{b}")
    # full-width rows (contiguous 1 KiB lines -> full HBM efficiency);
    # ScalarE carries no DMA descriptors at all: it is the exp engine and
    # must never be the resource the PE waits on
    for kk in range(CC):
        nc.sync.dma_start(st.face[:, kk, :], face8[b, kk])
        nc.sync.dma_start(st.audio[:, kk, :], audio8[b, kk])
    return st


def _emit_dma_facer(nc, inpool, io, st):
    # gpsimd queue: keeps the bf16 residual stream off the sync queue
    st.facer = inpool.tile([128, CC, N], BF16, tag="facer", name=f"facer{st.b}")
    for kk in range(CC):
        nc.gpsimd.dma_start(st.facer[:, kk, :], io["faceres"][st.b, kk])


def _emit(nc, tc, ctx, io):
    wpool = ctx.enter_context(tc.tile_pool(name="weights", bufs=1))
    inpool = ctx.enter_context(tc.tile_pool(name="inputs", bufs=2))
    qkpool = ctx.enter_context(tc.tile_pool(name="qk", bufs=2))
    vtpool = ctx.enter_context(tc.tile_pool(name="vt", bufs=2))
    ptpool = ctx.enter_context(tc.tile_pool(name="pt", bufs=2))
    misc = ctx.enter_context(tc.tile_pool(name="misc", bufs=2))
    tmppool = ctx.enter_context(tc.tile_pool(name="tmp", bufs=4))
    gps = ctx.enter_context(tc.tile_pool(name="gps", bufs=4, space="PSUM"))
    eps = ctx.enter_context(tc.tile_pool(name="eps", bufs=2, space="PSUM"))

    # --- persistent weights/constants ---
    # wqk first (the very first LDWEIGHTS needs it); wv is DMAed later,
    # between batch-0's face chunks, so the ~100GB/s sync queue delivers
    # the first-matmul inputs as early as possible
    wqk_sb = wpool.tile([128, CC, 256], FP8)
    nc.sync.dma_start(wqk_sb[:], io["wqk"][:])
    WQ_OFF, WK_OFF = 0, 128
    bq_sb = wpool.tile([128, 1], F32)
    nc.sync.dma_start(bq_sb[:], io["bq"][:])
    bk_sb = wpool.tile([128, 1], F32)
    nc.sync.dma_start(bk_sb[:], io["bk"][:])
    ones_mat = wpool.tile([128, 2, 128], FP8)
    nc.vector.memset(ones_mat[:], 1.0)

    # warm the ScalarE exp table off the critical path
    warm_ps = gps.tile([128, 1], F32, tag="g")
    warm_sb = wpool.tile([128, 1], F32)
    nc.vector.memset(warm_sb[:], 0.0)
    nc.scalar.activation(warm_ps[:], warm_sb[:], EXP)

    out = io["out"]

    def emit_qk_proj(st):
        """q/k projections: [128, 1024] (dup halves) = [W|W] @ x."""
        b = st.b
        st.q = qkpool.tile([128, N], BF16, tag="q", name=f"q{b}")
        st.k = qkpool.tile([128, N], BF16, tag="k", name=f"k{b}")
        for (w_off, x, dst, bias) in ((WQ_OFF, st.face, st.q, bq_sb),
                                      (WK_OFF, st.audio, st.k, bk_sb)):
            for j in range(NJ):
                p = gps.tile([128, 512], F32, tag="g", name=f"qkp{b}_{j}")
                for kk in range(0, CC, 2):
                    nc.tensor.matmul(p[:], wqk_sb[:, kk:kk + 2, ds(w_off, 128)],
                                     x[:, kk:kk + 2, ds(j * 512, 512)],
                                     start=(kk == 0), stop=(kk == CC - 2),
                                     perf_mode=DR)
                nc.vector.tensor_scalar_add(dst[:, ds(j * 512, 512)], p[:], bias[:])

    def emit_v_proj(st, ts):
        """v projection tiles ts, transposed: Vt[nk, c] (gamma pre-folded)."""
        b = st.b
        if not hasattr(st, "vt") or st.vt is None:
            st.vt = vtpool.tile([128, NT, C], FP8, tag="vt", name=f"vt{b}")
        for t in ts:
            vp = gps.tile([128, 512], F32, tag="g", name=f"vp{b}_{t}")
            for kk in range(0, CC, 2):
                nc.tensor.matmul(vp[:], st.audio[:, kk:kk + 2, ds(t * 128, 128)],
                                 wv_sb[:, kk:kk + 2, :],
                                 start=(kk == 0), stop=(kk == CC - 2),
                                 perf_mode=DR)
            if t % 2 == 0:
                nc.scalar.copy(st.vt[:, t, :], vp[:])
            else:
                nc.vector.tensor_scalar_mul(st.vt[:, t, :], vp[:], 1.0)

    def emit_energy_pair(st, t):
        """Energy tiles (t, t+1) + exp; row-packed pairs (K=64 each) run
        concurrently in disjoint halves of the PE array."""
        b = st.b
        if not hasattr(st, "pt") or st.pt is None:
            st.pt = ptpool.tile([128, NT, NJ, 512], FP8, tag="pt", name=f"pt{b}")
        ep = [eps.tile([128, NJ, 512], F32, tag="e", name=f"ep{b}_{t+h}")
              for h in range(2)]
        for j in range(NJ):
            for h in range(2):  # h=0 -> rows 0:64, h=1 -> rows 64:128
                hs = ds(h * 64, 64)
                nc.tensor.matmul(ep[h][:, j, :], st.k[hs, ds((t + h) * 128, 128)],
                                 st.q[hs, ds(j * 512, 512)], start=True, stop=True,
                                 tile_position=(h * 64, 0))
        for h in range(2):
            # PT = exp(ET/sqrt(64)); softmax shift-invariance => no max pass
            nc.scalar.activation(st.pt[:, t + h], ep[h][:], EXP, scale=0.125)

    def emit_sums(st):
        """Softmax denominators, pre-broadcast: S[p, nq] = sum_nk PT.
        Chains are de-interleaved so each half's reciprocal starts as soon
        as its own accumulation finishes."""
        b = st.b
        st.recip = misc.tile([128, N], F32, tag="recip", name=f"recip{b}")
        st.sp = [gps.tile([128, 512], F32, tag="g", name=f"sp{b}_{j}")
                 for j in range(NJ)]
        for j in range(NJ):
            for t in range(0, NT, 2):
                nc.tensor.matmul(st.sp[j][:], ones_mat[:], st.pt[:, t:t + 2, j],
                                 start=(t == 0), stop=(t == NT - 2), perf_mode=DR)
            nc.vector.reciprocal_approx_fast(st.recip[:, ds(j * 512, 512)],
                                             st.sp[j][:])

    def emit_recip(st):
        pass

    def emit_pv_cc(st, cc, last_batch):
        """PV + residual for one c-chunk:
        out[c, nq] = (gamma*O)/S + (face + gamma*bv)."""
        b = st.b
        op = [gps.tile([128, 512], F32, tag="g", name=f"op{b}_{cc}_{j}")
              for j in range(NJ)]
        for t in range(0, NT, 2):
            for j in range(NJ):
                nc.tensor.matmul(op[j][:], st.vt[:, t:t + 2, ds(cc * 128, 128)],
                                 st.pt[:, t:t + 2, j],
                                 start=(t == 0), stop=(t == NT - 2), perf_mode=DR)
        tmp = tmppool.tile([128, N], BF16, tag="tmp", name=f"tmp{b}_{cc}")
        for j in range(NJ):
            nc.vector.tensor_mul(tmp[:, ds(j * 512, 512)], op[j][:],
                                 st.recip[:, ds(j * 512, 512)])
        fslice = st.facer[:, cc, :]
        if last_batch:
            # VectorE per-half adds: minimal serial tail after the last MM
            for j in range(NJ):
                nc.vector.tensor_add(fslice[:, ds(j * 512, 512)],
                                     tmp[:, ds(j * 512, 512)],
                                     fslice[:, ds(j * 512, 512)])
                nc.sync.dma_start(out[b, cc, :, ds(j * 512, 512)],
                                  st.facer[:, cc, ds(j * 512, 512)])
        else:
            nc.gpsimd.tensor_add(fslice, tmp[:], fslice)
            nc.sync.dma_start(out[b, cc], fslice)

    # ---------------- pipelined emission ----------------
    # batch 0: the pipeline-fill critical path. Split the 1.25MB of
    # first-batch inputs across BOTH HW DMA queues so the q/k projections
    # and the energy matmuls can start ~10us earlier than a single queue
    # would allow.
    face8, audio8 = io["face8"], io["audio8"]
    st = _BatchState()
    st.b = 0
    st.face = inpool.tile([128, CC, N], FP8, tag="face", name="face0")
    st.audio = inpool.tile([128, CC, N], FP8, tag="audio", name="audio0")
    nc.sync.dma_start(st.face[:, 0, :], face8[0, 0])
    nc.sync.dma_start(st.face[:, 1, :], face8[0, 1])
    nc.scalar.dma_start(st.audio[:, 0, :], audio8[0, 0])
    nc.scalar.dma_start(st.audio[:, 1, :], audio8[0, 1])
    wv_sb = wpool.tile([128, CC, C], FP8)  # pre-scaled by gamma on host
    nc.sync.dma_start(wv_sb[:], io["wv"][:])
    nc.scalar.dma_start(st.face[:, 2, :], face8[0, 2])
    nc.scalar.dma_start(st.face[:, 3, :], face8[0, 3])
    nc.scalar.dma_start(st.audio[:, 2, :], audio8[0, 2])
    nc.scalar.dma_start(st.audio[:, 3, :], audio8[0, 3])
    _emit_dma_facer(nc, inpool, io, st)
    prev = None
    for b in range(BPC):
        nxt = _emit_dma_in(nc, inpool, io, b + 1) if b + 1 < BPC else None
        st.vt = None
        st.pt = None
        emit_qk_proj(st)
        if prev is None:
            # batch 0: no PV to interleave; spread energy pairs with v-proj
            # so the 2-slot exp PSUM pool never stalls the PE
            emit_energy_pair(st, 0)
            emit_v_proj(st, range(0, 4))
            emit_energy_pair(st, 2)
            emit_v_proj(st, range(4, 8))
            emit_energy_pair(st, 4)
            emit_energy_pair(st, 6)
        else:
            emit_v_proj(st, range(0, 8))
            emit_sums(prev)
            emit_recip(prev)
            emit_energy_pair(st, 0)
            emit_pv_cc(prev, 0, False)
            emit_energy_pair(st, 2)
            emit_pv_cc(prev, 1, False)
            emit_energy_pair(st, 4)
            emit_pv_cc(prev, 2, False)
            emit_energy_pair(st, 6)
            emit_pv_cc(prev, 3, False)
        # bf16 residual prefetch for the next batch, emitted AFTER this
        # iteration's out-DMAs so sync-queue order matches slot-free order
        if nxt is not None:
            _emit_dma_facer(nc, inpool, io, nxt)
        prev, st = st, nxt

    # drain: B-phase of the last batch
    emit_sums(prev)
    emit_recip(prev)
    for cc in range(CC):
        emit_pv_cc(prev, cc, True)


def _build_program():
    global _PROGRAM
    if _PROGRAM is not None:
        return _PROGRAM
    nc = bacc.Bacc("TRN2", target_bir_lowering=False, debug=False,
                   num_devices=N_CORES)
    d = {}
    d["face8"] = nc.dram_tensor("face8", [BPC, CC, 128, N], FP8, kind="ExternalInput").ap()
    d["audio8"] = nc.dram_tensor("audio8", [BPC, CC, 128, N], FP8, kind="ExternalInput").ap()
    d["faceres"] = nc.dram_tensor("faceres", [BPC, CC, 128, N], BF16, kind="ExternalInput").ap()
    d["wqk"] = nc.dram_tensor("wqk", [128, CC, 256], FP8, kind="ExternalInput").ap()
    d["wv"] = nc.dram_tensor("wv", [128, CC, C], FP8, kind="ExternalInput").ap()
    d["bq"] = nc.dram_tensor("bq", [128, 1], F32, kind="ExternalInput").ap()
    d["bk"] = nc.dram_tensor("bk", [128, 1], F32, kind="ExternalInput").ap()
    d["out"] = nc.dram_tensor("out", [BPC, CC, 128, N], BF16, kind="ExternalOutput").ap()

    with tile.TileContext(nc) as tc:
        with ExitStack() as ctx:
            _emit(nc, tc, ctx, d)
    nc.compile()
    _PROGRAM = nc
    return nc


def _make_in_maps(face_feat, audio_feat, Wq, bq, Wk, bk, Wv, bv, gamma):
    fp8 = ml_dtypes.float8_e4m3fn
    bf16 = ml_dtypes.bfloat16
    g = np.float32(np.asarray(gamma).reshape(-1)[0])

    face = np.ascontiguousarray(face_feat.reshape(B, C, N), dtype=np.float32)
    audio = np.ascontiguousarray(audio_feat.reshape(B, C, N), dtype=np.float32)

    # residual folds in gamma*bv (v-bias passes through softmax exactly)
    faceres = (face + (g * bv.astype(np.float32))[None, :, None])
    faceres = faceres.astype(bf16).reshape(B, CC, 128, N)

    face8 = face.astype(fp8).reshape(B, CC, 128, N)
    audio8 = audio.astype(fp8).reshape(B, CC, 128, N)

    def chunk_t(wT):  # [C, M] -> [128, CC, M]
        return np.ascontiguousarray(wT.reshape(CC, 128, -1).transpose(1, 0, 2))

    # q/k weights duplicated along M so projections emit both partition
    # halves (feeds the row-packed energy matmuls); gamma folded into Wv;
    # all three packed into one tensor for a single weights DMA
    wqT = chunk_t(np.concatenate([Wq.T, Wq.T], axis=1).astype(np.float32).astype(fp8))
    wkT = chunk_t(np.concatenate([Wk.T, Wk.T], axis=1).astype(np.float32).astype(fp8))
    wvT = np.ascontiguousarray(chunk_t((g * Wv.astype(np.float32)).T.astype(fp8)))
    wqk = np.ascontiguousarray(np.concatenate([wqT, wkT], axis=2))
    bq2 = np.tile(bq.astype(np.float32).reshape(CQK, 1), (2, 1))
    bk2 = np.tile(bk.astype(np.float32).reshape(CQK, 1), (2, 1))

    in_maps = []
    for i in range(N_CORES):
        sl = slice(i * BPC, (i + 1) * BPC)
        in_maps.append({
            "face8": face8[sl], "audio8": audio8[sl], "faceres": faceres[sl],
            "wqk": wqk, "wv": wvT, "bq": bq2, "bk": bk2,
        })
    return in_maps


def kernel(face_feat, audio_feat, Wq, bq, Wk, bk, Wv, bv, gamma):
    nc = _build_program()
    in_maps = _make_in_maps(face_feat, audio_feat, Wq, bq, Wk, bk, Wv, bv, gamma)
    res = run_bass_kernel_spmd(nc, in_maps, core_ids=list(range(N_CORES)))
    out = np.concatenate([res.results[i]["out"] for i in range(N_CORES)], axis=0)
    return out.astype(np.float32).reshape(B, C, H, W)
